# revision 8
# baseline (speedup 1.0000x reference)
"""Trainium2 Bass kernel for BoltzmannMoE (top-2 of 8 experts, N=8192, D=1024, H=4096, O=1024).

Strategy (balanced expert-parallel across 8 NeuronCores):
  - Host: gate (softmax -> top-2 -> renormalize) in numpy fp32.
  - Experts are split into two quads; each quad of cores serves 4 experts,
    each expert's tokens split evenly across the quad's 4 cores.  Pass p on
    every core runs its quad's rank-p expert, so all cores execute the same
    program (SPMD) with per-core weight/token inputs.  Slot sizes are
    rank-matched between the two quads, giving ~2057 tokens/core vs the
    2129-token max-expert count of plain one-expert-per-core.
  - Device (per core, per pass): y_e^T = W2_e^T @ relu(W1_e^T @ xg^T + b1_e)
    as two chained bf16 matmuls (full PE rate + fast weight load, ~3e-3 rel
    err vs the 2e-2 gate).  Weights are streamed per pass in 256-512KB tiles;
    the token block stays SBUF-resident.
"""

import numpy as np
import ml_dtypes

import concourse.bass as bass
import concourse.mybir as mybir
import concourse.tile as tile
from concourse import bacc
from concourse.bass_utils import run_bass_kernel_spmd

P = 128
D, H, O, E, KTOP = 1024, 4096, 1024, 8, 2
TEMP = 2.718281828459045
NCORES = 8
GROUP = 4          # cores (and experts) per quad
NPASS = E // 2     # one pass per rank within the quad

DK = D // P    # 8  k-subtiles for mm1
HK = H // P    # 32 k-subtiles for mm2
HT = H // P    # 32 h output tiles (mm1 M dim)
OT = O // P    # 8  o output tiles (mm2 M dim)

DT = mybir.dt.bfloat16
NPDT = ml_dtypes.bfloat16

LAST_RESULTS = None  # BassKernelResults of the most recent device run (for test harness)


def _subs(n):
    """Free-dim blocks for an n-token slot: single matmul when <=512, else
    two near-halves (kept >=256 so every MM runs near full issue rate)."""
    if n <= 512:
        return [(0, n)]
    assert n <= 1024
    h = (n + 1) // 2
    return [(0, h), (h, n - h)]


def _plan(counts):
    """Pack the 8 experts' token counts into the SPMD bin structure: 2 core
    quads x NPASS slots, slot p sized s_p on every core, each (core, slot)
    bin single-expert.  Within a quad, slot p's four bins are distributed
    among the quad's experts (column sums = 4); an expert may take several
    bins of one slot and bins of several slots (4 bins total).  Minimizes
    C = sum(s_p) by exact search (floor: ceil(total/8)); falls back to the
    rank-matched diagonal plan.  Returns (sizes, bins) with bins[core][p] =
    (expert, start, cnt): tokens idxs[e][start:start+cnt]."""
    from itertools import combinations

    counts = [int(c) for c in counts]

    # --- diagonal plan (always feasible): quad rank-p expert owns slot p ---
    best = None
    for sub in combinations(range(1, E), GROUP - 1):
        g1 = sorted([0, *sub], key=lambda e: -counts[e])
        g2 = sorted([e for e in range(E) if e not in g1], key=lambda e: -counts[e])
        sizes = [
            max(16, -(-max(counts[g1[p]], counts[g2[p]]) // GROUP))
            for p in range(GROUP)
        ]
        if best is None or sum(sizes) < best[0]:
            best = (sum(sizes), [g1, g2], sizes)
    diag_c, groups, sizes = best
    diag_rows = [tuple(GROUP if q == p else 0 for q in range(NPASS)) for p in range(NPASS)]
    plan = (sizes, [(groups[0], diag_rows), (groups[1], diag_rows)])

    try:
        ex = _plan_exact(counts, diag_c)
        if ex is not None:
            plan = ex
    except Exception:
        pass

    sizes, gplans = plan
    bins = [[None] * NPASS for _ in range(NCORES)]
    for g, (grp, rows) in enumerate(gplans):
        for p in range(NPASS):
            occ = []
            for e, row in zip(grp, rows):
                occ += [e] * row[p]
            assert len(occ) == GROUP
            for j, e in enumerate(occ):
                bins[g * GROUP + j][p] = [e, 0, 0]
        for e, row in zip(grp, rows):
            rem, cur = counts[e], 0
            for p in range(NPASS):
                for j in range(GROUP):
                    b = bins[g * GROUP + j][p]
                    if b[0] == e:
                        take = min(sizes[p], rem)
                        b[1], b[2] = cur, take
                        cur += take
                        rem -= take
            assert rem == 0, (e, rem)
    return sizes, [[tuple(b) for b in row] for row in bins]


def _plan_exact(counts, diag_c, max_t=9, eps_range=40):
    """Search slot sizes + per-quad assignment matrices reaching C below the
    diagonal plan's.  Rows are restricted to 4 bins per expert (other row
    sums are infeasible while slot sizes stay near C/4)."""
    from itertools import combinations, product

    cnt = np.asarray(counts)
    rows = np.array(
        [r for r in product(range(GROUP + 1), repeat=NPASS) if sum(r) == GROUP],
        dtype=np.int64,
    )

    splits = []
    for sub in combinations(range(1, E), GROUP - 1):
        g1 = [0, *sub]
        g2 = [e for e in range(E) if e not in g1]
        splits.append((int(max(cnt[g1].sum(), cnt[g2].sum())), g1, g2))
    splits.sort()

    total = int(cnt.sum())
    base_c = -(-total // NCORES)
    base = base_c // NPASS

    for t in range(max_t):
        C = base_c + t
        if C >= diag_c:
            break
        tgt = C - NPASS * base
        r0 = np.arange(-eps_range, eps_range + 1)
        e0, e1, e2 = np.meshgrid(r0, r0, r0, indexing="ij")
        e3 = tgt - (e0 + e1 + e2)
        ok = (np.abs(e3) <= eps_range) & (e0 >= e1) & (e1 >= e2) & (e2 >= e3)
        s_all = np.stack([e0[ok], e1[ok], e2[ok], e3[ok]], axis=1) + base
        if len(s_all) == 0:
            continue
        caps = s_all @ rows.T

        for msum, g1, g2 in splits:
            slack1 = GROUP * C - int(cnt[g1].sum())
            slack2 = GROUP * C - int(cnt[g2].sum())
            if slack1 < 0 or slack2 < 0:
                continue
            feas = np.ones(len(s_all), dtype=bool)
            for e, slack in [(e, slack1) for e in g1] + [(e, slack2) for e in g2]:
                o = caps - counts[e]
                feas &= ((o >= 0) & (o <= slack)).any(axis=1)
                if not feas.any():
                    break
            if not feas.any():
                continue
            for mi in np.nonzero(feas)[0]:
                s = s_all[mi]
                m1 = _dfs_group([counts[e] for e in g1], s, rows, slack1)
                if m1 is None:
                    continue
                m2 = _dfs_group([counts[e] for e in g2], s, rows, slack2)
                if m2 is None:
                    continue
                return [int(v) for v in s], [(g1, m1), (g2, m2)]
    return None


def _dfs_group(cnts, s, rows, slack):
    caps = rows @ s
    cand = []
    for c in cnts:
        o = caps - c
        sel = np.nonzero((o >= 0) & (o <= slack))[0]
        if len(sel) == 0:
            return None
        sel = sel[np.argsort(o[sel])]
        cand.append([(rows[i], int(o[i])) for i in sel])

    res = [None] * len(cnts)

    def dfs(d, colsum, slack_left):
        if d == len(cnts):
            return all(cs == GROUP for cs in colsum)
        for row, o in cand[d]:
            if o > slack_left:
                continue
            ncs = [colsum[p] + int(row[p]) for p in range(NPASS)]
            if any(c > GROUP for c in ncs):
                continue
            res[d] = row
            if dfs(d + 1, ncs, slack_left - o):
                return True
        return False

    if dfs(0, [0] * NPASS, slack):
        return [tuple(int(v) for v in r) for r in res]
    return None


def _build_program(sizes):
    nc = bacc.Bacc("TRN2", target_bir_lowering=False, debug=False)
    C = sum(sizes)
    offs = [sum(sizes[:p]) for p in range(len(sizes))]

    xgT = nc.dram_tensor("xgT", (P, DK, C), DT, kind="ExternalInput")
    w1 = nc.dram_tensor("w1", (NPASS, HT, P, DK, P), DT, kind="ExternalInput")
    w2 = nc.dram_tensor("w2", (NPASS, OT, 2, P, HK // 2, P), DT, kind="ExternalInput")
    b1 = nc.dram_tensor("b1", (P, NPASS * HT), mybir.dt.float32, kind="ExternalInput")
    yT = nc.dram_tensor("yT", (P, OT, C), mybir.dt.float32, kind="ExternalOutput")

    smax = max(sizes)

    with tile.TileContext(nc) as tc:
        with (
            tc.tile_pool(name="const", bufs=1) as const,
            tc.tile_pool(name="ht", bufs=2) as ht_pool,
            tc.tile_pool(name="w1p", bufs=6) as w1_pool,
            tc.tile_pool(name="w2p", bufs=4) as w2_pool,
            tc.tile_pool(name="yst", bufs=3) as yst_pool,
            tc.tile_pool(name="psa", bufs=4, space="PSUM") as psa,
            tc.tile_pool(name="psb", bufs=4, space="PSUM") as psb,
        ):
            # PE warmup: the HAM clock gate holds the PE at 1.2 GHz until it
            # has been busy for a ~3.4us window.  A burst of dummy matmuls on
            # a memset tile during the opening DMAs releases the throttle, so
            # the real matmuls start dense AND at full clock.
            wu_x = const.tile([P, 512], DT, name="wu_x")
            nc.vector.memset(wu_x[:], 0.0)
            wu_ps = psb.tile([P, 512], mybir.dt.float32, name="ps_b")
            nwu = 14
            for i in range(nwu):
                nc.tensor.matmul(
                    wu_ps[:],
                    wu_x[:, :P],
                    wu_x[:],
                    start=(i == 0),
                    stop=(i == nwu - 1),
                    skip_group_check=True,
                )

            xg_sb = const.tile([P, DK, C], DT, name="xg_sb")
            # Critical-path startup DMAs, ordered so the first matmul chain is
            # gated on minimal data (each trigger costs ~0.7us of SP issue
            # time): first h-tile of W1, then pass-0 tokens in k-quarters.
            # Later passes' xg slices are issued during the preceding pass's
            # mm2.
            npre = 6
            w1_pre = [w1_pool.tile([P, DK, P], DT, name="w1_t") for _ in range(npre)]
            nc.sync.dma_start(w1_pre[0][:], w1.ap()[0, 0])
            for kq in range(4):
                k0 = kq * (DK // 4)
                nc.sync.dma_start(
                    xg_sb[:, k0 : k0 + DK // 4, : sizes[0]],
                    xgT.ap()[:, k0 : k0 + DK // 4, : sizes[0]],
                )
            b1_sb = const.tile([P, NPASS * HT], mybir.dt.float32)
            nc.sync.dma_start(b1_sb[:], b1.ap())
            for ht in range(1, npre):
                nc.sync.dma_start(w1_pre[ht][:], w1.ap()[0, ht])

            for p, psize in enumerate(sizes):
                off = offs[p]
                subs = _subs(psize)

                ht_t = ht_pool.tile([P, HK, smax], DT, name="ht_t")

                # ---- mm1: hT = relu(W1^T @ xgT + b1) ----
                for ht in range(HT):
                    if p == 0 and ht < npre:
                        w1_t = w1_pre[ht]
                    else:
                        w1_t = w1_pool.tile([P, DK, P], DT, name="w1_t")
                        nc.sync.dma_start(w1_t[:], w1.ap()[p, ht])
                    pss = [
                        psa.tile([P, 512], mybir.dt.float32, name="ps_a")
                        for _ in subs
                    ]
                    # k-outer / sub-inner: the stationary operand is reused by
                    # consecutive matmuls, halving weight-load pressure
                    for k in range(DK):
                        for si, (s0, sz) in enumerate(subs):
                            nc.tensor.matmul(
                                pss[si][:, :sz],
                                w1_t[:, k, :],
                                xg_sb[:, k, off + s0 : off + s0 + sz],
                                start=(k == 0),
                                stop=(k == DK - 1),
                                skip_group_check=True,
                            )
                    for si, (s0, sz) in enumerate(subs):
                        nc.scalar.activation(
                            ht_t[:, ht, s0 : s0 + sz],
                            pss[si][:, :sz],
                            mybir.ActivationFunctionType.Relu,
                            bias=b1_sb[:, p * HT + ht : p * HT + ht + 1],
                        )

                # ---- mm2: yT = W2^T @ hT ----
                for ot in range(OT):
                    w2_ts = []
                    for hf in range(2):
                        w2_t = w2_pool.tile([P, HK // 2, P], DT, name="w2_t")
                        nc.sync.dma_start(w2_t[:], w2.ap()[p, ot, hf])
                        w2_ts.append(w2_t)
                    # next pass's xg block, one k-slice per ot iteration
                    if p + 1 < len(sizes) and ot < DK:
                        noff = offs[p + 1]
                        nc.sync.dma_start(
                            xg_sb[:, ot, noff : noff + sizes[p + 1]],
                            xgT.ap()[:, ot, noff : noff + sizes[p + 1]],
                        )
                    pbs = [
                        psb.tile([P, 512], mybir.dt.float32, name="ps_b")
                        for _ in subs
                    ]
                    for k in range(HK):
                        w2_t = w2_ts[k // (HK // 2)]
                        for si, (s0, sz) in enumerate(subs):
                            nc.tensor.matmul(
                                pbs[si][:, :sz],
                                w2_t[:, k % (HK // 2), :],
                                ht_t[:, k, s0 : s0 + sz],
                                start=(k == 0),
                                stop=(k == HK - 1),
                                skip_group_check=True,
                            )
                    for si, (s0, sz) in enumerate(subs):
                        st = yst_pool.tile([P, 512], mybir.dt.float32, name="y_st")
                        nc.vector.tensor_copy(st[:, :sz], pbs[si][:, :sz])
                        nc.scalar.dma_start(
                            yT.ap()[:, ot, off + s0 : off + s0 + sz], st[:, :sz]
                        )

    nc.compile()
    return nc


def _host_gate(x, Wg, bg):
    """Replicates reference gating in fp32: softmax(scores/T) -> top-2 -> renorm."""
    scores = (x @ Wg + bg) / np.float32(TEMP)
    m = scores.max(axis=-1, keepdims=True)
    un = np.exp(scores - m)
    probs = un / un.sum(-1, keepdims=True)
    order = np.argsort(-probs, axis=1, kind="stable")[:, :KTOP]
    vals = np.take_along_axis(probs, order, axis=1)
    w = np.zeros_like(probs)
    np.put_along_axis(w, order, vals, axis=1)
    w = w / (w.sum(-1, keepdims=True) + np.float32(1e-8))
    return w


def kernel(x, Wg, bg, W1, b1, W2, b2):
    global LAST_RESULTS
    x = np.ascontiguousarray(np.asarray(x, dtype=np.float32))
    Wg = np.asarray(Wg, dtype=np.float32)
    bg = np.asarray(bg, dtype=np.float32)
    W1 = np.asarray(W1, dtype=np.float32)
    b1 = np.asarray(b1, dtype=np.float32)
    W2 = np.asarray(W2, dtype=np.float32)
    b2 = np.asarray(b2, dtype=np.float32)
    N = x.shape[0]

    w = _host_gate(x, Wg, bg)  # [N, E] sparse renormalized top-2 weights

    idxs, counts = [], []
    for e in range(E):
        idx = np.nonzero(w[:, e])[0]
        idxs.append(idx)
        counts.append(len(idx))

    sizes, bins = _plan(counts)
    C = sum(sizes)
    offs = [sum(sizes[:p]) for p in range(len(sizes))]

    # per-expert device layouts (bf16), referenced by every core that uses them
    x_bf = x.astype(NPDT)
    w1_pm, w2_pm, b1_pm = {}, {}, {}
    for e in range(E):
        w1_pm[e] = np.ascontiguousarray(
            W1[e].astype(NPDT).reshape(DK, P, HT, P).transpose(2, 1, 0, 3)
        )
        w2_pm[e] = np.ascontiguousarray(
            W2[e].astype(NPDT).reshape(2, HK // 2, P, OT, P).transpose(3, 0, 2, 1, 4)
        )
        b1_pm[e] = np.ascontiguousarray(b1[e].reshape(HT, P).T)

    # chunk bookkeeping: (core, pass) -> (expert, token index array)
    chunk_idx = [
        [(e, idxs[e][start : start + cnt]) for (e, start, cnt) in bins[core]]
        for core in range(NCORES)
    ]

    in_maps = []
    w_cache = {}
    for core in range(NCORES):
        xg = np.zeros((C, D), dtype=NPDT)
        for p in range(NPASS):
            e, idx = chunk_idx[core][p]
            xg[offs[p] : offs[p] + len(idx)] = x_bf[idx]
        xgT = np.ascontiguousarray(xg.T.reshape(DK, P, C).transpose(1, 0, 2))
        eseq = tuple(e for e, _, _ in bins[core])
        if eseq not in w_cache:
            w_cache[eseq] = (
                np.stack([w1_pm[e] for e in eseq]),
                np.stack([w2_pm[e] for e in eseq]),
                np.concatenate([b1_pm[e] for e in eseq], axis=1).astype(np.float32),
            )
        cw1, cw2, cb1 = w_cache[eseq]
        in_maps.append({"xgT": xgT, "w1": cw1, "w2": cw2, "b1": cb1})

    nc = _build_program(sizes)
    res = None
    last_exc = None
    for attempt in range(3):
        try:
            res = run_bass_kernel_spmd(nc, in_maps, core_ids=list(range(NCORES)))
            break
        except Exception as exc:  # device wedge under profiling is transient
            last_exc = exc
            try:
                import jax

                jax.clear_caches()
            except Exception:
                pass
    if res is None:
        raise last_exc
    LAST_RESULTS = res

    out = np.zeros((N, O), dtype=np.float32)
    for core in range(NCORES):
        yT = res.results[core]["yT"]  # [P, OT, C]
        y_all = yT.transpose(1, 0, 2).reshape(O, C)
        for p in range(NPASS):
            e, idx = chunk_idx[core][p]
            if len(idx) == 0:
                continue
            y = y_all[:, offs[p] : offs[p] + len(idx)].T  # [cnt, O]
            out[idx] += w[idx, e][:, None] * (y + b2[e])
    return out


# revision 12
# speedup vs baseline: 1.0261x; 1.0261x over previous
"""Trainium2 Bass kernel for BoltzmannMoE (top-2 of 8 experts, N=8192, D=1024, H=4096, O=1024).

Strategy (balanced expert-parallel across 8 NeuronCores):
  - Host: gate (softmax -> top-2 -> renormalize) in numpy fp32.
  - Experts are split into two quads; each quad of cores serves 4 experts,
    each expert's tokens split evenly across the quad's 4 cores.  Pass p on
    every core runs its quad's rank-p expert, so all cores execute the same
    program (SPMD) with per-core weight/token inputs.  Slot sizes are
    rank-matched between the two quads, giving ~2057 tokens/core vs the
    2129-token max-expert count of plain one-expert-per-core.
  - Device (per core, per pass): y_e^T = W2_e^T @ relu(W1_e^T @ xg^T + b1_e)
    as two chained bf16 matmuls (full PE rate + fast weight load, ~3e-3 rel
    err vs the 2e-2 gate).  Weights are streamed per pass in 256-512KB tiles;
    the token block stays SBUF-resident.
"""

import numpy as np
import ml_dtypes

import concourse.bass as bass
import concourse.mybir as mybir
import concourse.tile as tile
from concourse import bacc
from concourse.bass_utils import run_bass_kernel_spmd

P = 128
D, H, O, E, KTOP = 1024, 4096, 1024, 8, 2
TEMP = 2.718281828459045
NCORES = 8
GROUP = 4          # cores (and experts) per quad
NPASS = E // 2     # one pass per rank within the quad

DK = D // P    # 8  k-subtiles for mm1
HK = H // P    # 32 k-subtiles for mm2
HT = H // P    # 32 h output tiles (mm1 M dim)
OT = O // P    # 8  o output tiles (mm2 M dim)

DT = mybir.dt.bfloat16
NPDT = ml_dtypes.bfloat16

LAST_RESULTS = None  # BassKernelResults of the most recent device run (for test harness)


def _subs(n):
    """Free-dim blocks for an n-token slot: single matmul when <=512, else
    two near-halves (kept >=256 so every MM runs near full issue rate)."""
    if n <= 512:
        return [(0, n)]
    assert n <= 1024
    h = (n + 1) // 2
    return [(0, h), (h, n - h)]


def _plan(counts):
    """Pack the 8 experts' token counts into the SPMD bin structure: 2 core
    quads x NPASS slots, slot p sized s_p on every core, each (core, slot)
    bin single-expert.  Within a quad, slot p's four bins are distributed
    among the quad's experts (column sums = 4); an expert may take several
    bins of one slot and bins of several slots (4 bins total).  Minimizes
    C = sum(s_p) by exact search (floor: ceil(total/8)); falls back to the
    rank-matched diagonal plan.  Returns (sizes, bins) with bins[core][p] =
    (expert, start, cnt): tokens idxs[e][start:start+cnt]."""
    from itertools import combinations

    counts = [int(c) for c in counts]

    # --- diagonal plan (always feasible): quad rank-p expert owns slot p ---
    best = None
    for sub in combinations(range(1, E), GROUP - 1):
        g1 = sorted([0, *sub], key=lambda e: -counts[e])
        g2 = sorted([e for e in range(E) if e not in g1], key=lambda e: -counts[e])
        sizes = [
            max(16, -(-max(counts[g1[p]], counts[g2[p]]) // GROUP))
            for p in range(GROUP)
        ]
        if best is None or sum(sizes) < best[0]:
            best = (sum(sizes), [g1, g2], sizes)
    diag_c, groups, sizes = best
    diag_rows = [tuple(GROUP if q == p else 0 for q in range(NPASS)) for p in range(NPASS)]
    plan = (sizes, [(groups[0], diag_rows), (groups[1], diag_rows)])

    try:
        diag_key = diag_c + 13 * sum(1 for s in sizes if s > 512)
        ex = _plan_exact(counts, diag_c, diag_key=diag_key)
        if ex is not None:
            plan = ex
    except Exception:
        pass

    sizes, gplans = plan
    bins = [[None] * NPASS for _ in range(NCORES)]
    for g, (grp, rows) in enumerate(gplans):
        for p in range(NPASS):
            occ = []
            for e, row in zip(grp, rows):
                occ += [e] * row[p]
            assert len(occ) == GROUP
            for j, e in enumerate(occ):
                bins[g * GROUP + j][p] = [e, 0, 0]
        for e, row in zip(grp, rows):
            rem, cur = counts[e], 0
            for p in range(NPASS):
                for j in range(GROUP):
                    b = bins[g * GROUP + j][p]
                    if b[0] == e:
                        take = min(sizes[p], rem)
                        b[1], b[2] = cur, take
                        cur += take
                        rem -= take
            assert rem == 0, (e, rem)
    return sizes, [[tuple(b) for b in row] for row in bins]


def _plan_exact(counts, diag_c, diag_key=None, pen=13, lo=460, hi=580):
    """Search slot sizes + per-quad assignment matrices minimizing the
    effective cost key = C + pen * (#slots > 512): each slot above 512 splits
    into two sub-blocks, adding one matmul per k-step (~pen tokens worth of
    issue overhead).  Rows are restricted to 4 bins per expert (other row
    sums are infeasible while slot sizes stay near C/4)."""
    from itertools import combinations, product

    cnt = np.asarray(counts)
    rows = np.array(
        [r for r in product(range(GROUP + 1), repeat=NPASS) if sum(r) == GROUP],
        dtype=np.int64,
    )

    splits = []
    for sub in combinations(range(1, E), GROUP - 1):
        g1 = [0, *sub]
        g2 = [e for e in range(E) if e not in g1]
        splits.append((int(max(cnt[g1].sum(), cnt[g2].sum())), g1, g2))
    splits.sort()

    total = int(cnt.sum())
    base_c = -(-total // NCORES)
    if diag_key is None:
        diag_key = diag_c + pen * NPASS

    for key in range(base_c, diag_key):
        for v in range(NPASS):
            C = key - pen * v
            if C < base_c or C >= diag_c:
                continue
            r0 = np.arange(lo, hi + 1)
            e0, e1, e2 = np.meshgrid(r0, r0, r0, indexing="ij")
            e3 = C - (e0 + e1 + e2)
            ok = (e3 >= lo) & (e3 <= hi) & (e0 >= e1) & (e1 >= e2) & (e2 >= e3)
            s_all = np.stack([e0[ok], e1[ok], e2[ok], e3[ok]], axis=1)
            s_all = s_all[(s_all > 512).sum(axis=1) == v]
            if len(s_all) == 0:
                continue
            caps = s_all @ rows.T

            for msum, g1, g2 in splits:
                slack1 = GROUP * C - int(cnt[g1].sum())
                slack2 = GROUP * C - int(cnt[g2].sum())
                if slack1 < 0 or slack2 < 0:
                    continue
                feas = np.ones(len(s_all), dtype=bool)
                for e, slack in [(e, slack1) for e in g1] + [(e, slack2) for e in g2]:
                    o = caps - counts[e]
                    feas &= ((o >= 0) & (o <= slack)).any(axis=1)
                    if not feas.any():
                        break
                if not feas.any():
                    continue
                for mi in np.nonzero(feas)[0]:
                    s = s_all[mi]
                    m1 = _dfs_group([counts[e] for e in g1], s, rows, slack1)
                    if m1 is None:
                        continue
                    m2 = _dfs_group([counts[e] for e in g2], s, rows, slack2)
                    if m2 is None:
                        continue
                    return [int(x) for x in s], [(g1, m1), (g2, m2)]
    return None


def _dfs_group(cnts, s, rows, slack):
    caps = rows @ s
    cand = []
    for c in cnts:
        o = caps - c
        sel = np.nonzero((o >= 0) & (o <= slack))[0]
        if len(sel) == 0:
            return None
        sel = sel[np.argsort(o[sel])]
        cand.append([(rows[i], int(o[i])) for i in sel])

    res = [None] * len(cnts)

    def dfs(d, colsum, slack_left):
        if d == len(cnts):
            return all(cs == GROUP for cs in colsum)
        for row, o in cand[d]:
            if o > slack_left:
                continue
            ncs = [colsum[p] + int(row[p]) for p in range(NPASS)]
            if any(c > GROUP for c in ncs):
                continue
            res[d] = row
            if dfs(d + 1, ncs, slack_left - o):
                return True
        return False

    if dfs(0, [0] * NPASS, slack):
        return [tuple(int(v) for v in r) for r in res]
    return None


def _build_program(sizes):
    nc = bacc.Bacc("TRN2", target_bir_lowering=False, debug=False)
    C = sum(sizes)
    offs = [sum(sizes[:p]) for p in range(len(sizes))]

    xgT = nc.dram_tensor("xgT", (P, DK, C), DT, kind="ExternalInput")
    w1 = nc.dram_tensor("w1", (NPASS, HT, P, DK, P), DT, kind="ExternalInput")
    w2 = nc.dram_tensor("w2", (NPASS, OT, 2, P, HK // 2, P), DT, kind="ExternalInput")
    b1 = nc.dram_tensor("b1", (P, NPASS * HT), mybir.dt.float32, kind="ExternalInput")
    yT = nc.dram_tensor("yT", (P, OT, C), mybir.dt.float32, kind="ExternalOutput")

    smax = max(sizes)

    with tile.TileContext(nc) as tc:
        with (
            tc.tile_pool(name="const", bufs=1) as const,
            tc.tile_pool(name="ht", bufs=2) as ht_pool,
            tc.tile_pool(name="w1p", bufs=6) as w1_pool,
            tc.tile_pool(name="w2p", bufs=4) as w2_pool,
            tc.tile_pool(name="yst", bufs=3) as yst_pool,
            tc.tile_pool(name="psa", bufs=4, space="PSUM") as psa,
            tc.tile_pool(name="psb", bufs=4, space="PSUM") as psb,
        ):
            # PE warmup: the HAM clock gate holds the PE at 1.2 GHz until it
            # has been busy for a ~3.4us window.  A burst of dummy matmuls on
            # a memset tile during the opening DMAs releases the throttle, so
            # the real matmuls start dense AND at full clock.
            wu_x = const.tile([P, 512], DT, name="wu_x")
            nc.vector.memset(wu_x[:], 0.0)
            wu_ps = psb.tile([P, 512], mybir.dt.float32, name="ps_b")
            nwu = 14
            for i in range(nwu):
                nc.tensor.matmul(
                    wu_ps[:],
                    wu_x[:, :P],
                    wu_x[:],
                    start=(i == 0),
                    stop=(i == nwu - 1),
                    skip_group_check=True,
                )

            xg_sb = const.tile([P, DK, C], DT, name="xg_sb")
            # Critical-path startup DMAs, ordered so the first matmul chain is
            # gated on minimal data (each trigger costs ~0.7us of SP issue
            # time): first h-tile of W1, then pass-0 tokens in k-quarters.
            # Later passes' xg slices are issued during the preceding pass's
            # mm2.
            npre = 6
            w1_pre = [w1_pool.tile([P, DK, P], DT, name="w1_t") for _ in range(npre)]
            nc.sync.dma_start(w1_pre[0][:], w1.ap()[0, 0])
            for kq in range(4):
                k0 = kq * (DK // 4)
                nc.sync.dma_start(
                    xg_sb[:, k0 : k0 + DK // 4, : sizes[0]],
                    xgT.ap()[:, k0 : k0 + DK // 4, : sizes[0]],
                )
            b1_sb = const.tile([P, NPASS * HT], mybir.dt.float32)
            nc.sync.dma_start(b1_sb[:], b1.ap())
            for ht in range(1, npre):
                nc.sync.dma_start(w1_pre[ht][:], w1.ap()[0, ht])

            for p, psize in enumerate(sizes):
                off = offs[p]
                subs = _subs(psize)

                ht_t = ht_pool.tile([P, HK, smax], DT, name="ht_t")

                # ---- mm1: hT = relu(W1^T @ xgT + b1) ----
                for ht in range(HT):
                    if p == 0 and ht < npre:
                        w1_t = w1_pre[ht]
                    else:
                        w1_t = w1_pool.tile([P, DK, P], DT, name="w1_t")
                        nc.sync.dma_start(w1_t[:], w1.ap()[p, ht])
                    pss = [
                        psa.tile([P, 512], mybir.dt.float32, name="ps_a")
                        for _ in subs
                    ]
                    # k-outer / sub-inner: the stationary operand is reused by
                    # consecutive matmuls, halving weight-load pressure
                    for k in range(DK):
                        for si, (s0, sz) in enumerate(subs):
                            nc.tensor.matmul(
                                pss[si][:, :sz],
                                w1_t[:, k, :],
                                xg_sb[:, k, off + s0 : off + s0 + sz],
                                start=(k == 0),
                                stop=(k == DK - 1),
                                skip_group_check=True,
                            )
                    for si, (s0, sz) in enumerate(subs):
                        nc.scalar.activation(
                            ht_t[:, ht, s0 : s0 + sz],
                            pss[si][:, :sz],
                            mybir.ActivationFunctionType.Relu,
                            bias=b1_sb[:, p * HT + ht : p * HT + ht + 1],
                        )

                # ---- mm2: yT = W2^T @ hT ----
                for ot in range(OT):
                    w2_ts = []
                    for hf in range(2):
                        w2_t = w2_pool.tile([P, HK // 2, P], DT, name="w2_t")
                        nc.sync.dma_start(w2_t[:], w2.ap()[p, ot, hf])
                        w2_ts.append(w2_t)
                    # next pass's xg block, one k-slice per ot iteration
                    if p + 1 < len(sizes) and ot < DK:
                        noff = offs[p + 1]
                        nc.sync.dma_start(
                            xg_sb[:, ot, noff : noff + sizes[p + 1]],
                            xgT.ap()[:, ot, noff : noff + sizes[p + 1]],
                        )
                    pbs = [
                        psb.tile([P, 512], mybir.dt.float32, name="ps_b")
                        for _ in subs
                    ]
                    for k in range(HK):
                        w2_t = w2_ts[k // (HK // 2)]
                        for si, (s0, sz) in enumerate(subs):
                            nc.tensor.matmul(
                                pbs[si][:, :sz],
                                w2_t[:, k % (HK // 2), :],
                                ht_t[:, k, s0 : s0 + sz],
                                start=(k == 0),
                                stop=(k == HK - 1),
                                skip_group_check=True,
                            )
                    for si, (s0, sz) in enumerate(subs):
                        st = yst_pool.tile([P, 512], mybir.dt.float32, name="y_st")
                        nc.vector.tensor_copy(st[:, :sz], pbs[si][:, :sz])
                        nc.scalar.dma_start(
                            yT.ap()[:, ot, off + s0 : off + s0 + sz], st[:, :sz]
                        )

    nc.compile()
    return nc


def _host_gate(x, Wg, bg):
    """Replicates reference gating in fp32: softmax(scores/T) -> top-2 -> renorm."""
    scores = (x @ Wg + bg) / np.float32(TEMP)
    m = scores.max(axis=-1, keepdims=True)
    un = np.exp(scores - m)
    probs = un / un.sum(-1, keepdims=True)
    order = np.argsort(-probs, axis=1, kind="stable")[:, :KTOP]
    vals = np.take_along_axis(probs, order, axis=1)
    w = np.zeros_like(probs)
    np.put_along_axis(w, order, vals, axis=1)
    w = w / (w.sum(-1, keepdims=True) + np.float32(1e-8))
    return w


def kernel(x, Wg, bg, W1, b1, W2, b2):
    global LAST_RESULTS
    x = np.ascontiguousarray(np.asarray(x, dtype=np.float32))
    Wg = np.asarray(Wg, dtype=np.float32)
    bg = np.asarray(bg, dtype=np.float32)
    W1 = np.asarray(W1, dtype=np.float32)
    b1 = np.asarray(b1, dtype=np.float32)
    W2 = np.asarray(W2, dtype=np.float32)
    b2 = np.asarray(b2, dtype=np.float32)
    N = x.shape[0]

    w = _host_gate(x, Wg, bg)  # [N, E] sparse renormalized top-2 weights

    idxs, counts = [], []
    for e in range(E):
        idx = np.nonzero(w[:, e])[0]
        idxs.append(idx)
        counts.append(len(idx))

    sizes, bins = _plan(counts)
    C = sum(sizes)
    offs = [sum(sizes[:p]) for p in range(len(sizes))]

    # per-expert device layouts (bf16), referenced by every core that uses them
    x_bf = x.astype(NPDT)
    w1_pm, w2_pm, b1_pm = {}, {}, {}
    for e in range(E):
        w1_pm[e] = np.ascontiguousarray(
            W1[e].astype(NPDT).reshape(DK, P, HT, P).transpose(2, 1, 0, 3)
        )
        w2_pm[e] = np.ascontiguousarray(
            W2[e].astype(NPDT).reshape(2, HK // 2, P, OT, P).transpose(3, 0, 2, 1, 4)
        )
        b1_pm[e] = np.ascontiguousarray(b1[e].reshape(HT, P).T)

    # chunk bookkeeping: (core, pass) -> (expert, token index array)
    chunk_idx = [
        [(e, idxs[e][start : start + cnt]) for (e, start, cnt) in bins[core]]
        for core in range(NCORES)
    ]

    in_maps = []
    w_cache = {}
    for core in range(NCORES):
        xg = np.zeros((C, D), dtype=NPDT)
        for p in range(NPASS):
            e, idx = chunk_idx[core][p]
            xg[offs[p] : offs[p] + len(idx)] = x_bf[idx]
        xgT = np.ascontiguousarray(xg.T.reshape(DK, P, C).transpose(1, 0, 2))
        eseq = tuple(e for e, _, _ in bins[core])
        if eseq not in w_cache:
            w_cache[eseq] = (
                np.stack([w1_pm[e] for e in eseq]),
                np.stack([w2_pm[e] for e in eseq]),
                np.concatenate([b1_pm[e] for e in eseq], axis=1).astype(np.float32),
            )
        cw1, cw2, cb1 = w_cache[eseq]
        in_maps.append({"xgT": xgT, "w1": cw1, "w2": cw2, "b1": cb1})

    nc = _build_program(sizes)
    res = None
    last_exc = None
    for attempt in range(3):
        try:
            res = run_bass_kernel_spmd(nc, in_maps, core_ids=list(range(NCORES)))
            break
        except Exception as exc:  # device wedge under profiling is transient
            last_exc = exc
            try:
                import jax

                jax.clear_caches()
            except Exception:
                pass
    if res is None:
        raise last_exc
    LAST_RESULTS = res

    out = np.zeros((N, O), dtype=np.float32)
    for core in range(NCORES):
        yT = res.results[core]["yT"]  # [P, OT, C]
        y_all = yT.transpose(1, 0, 2).reshape(O, C)
        for p in range(NPASS):
            e, idx = chunk_idx[core][p]
            if len(idx) == 0:
                continue
            y = y_all[:, offs[p] : offs[p] + len(idx)].T  # [cnt, O]
            out[idx] += w[idx, e][:, None] * (y + b2[e])
    return out
